# revision 1
# baseline (speedup 1.0000x reference)
"""Trainium2 Bass kernel for nn_CVKANTimeSeries.

Reference computation (per batch element b, sequence s, channel d):
  - complex embedding zr/zi = x @ er_w/ei_w + bias, rotated by positional
    phases (cos/sin tables).
  - 4 stacked "polarizing" layers: causal cumulative mean -> magnitude/phase
    -> tiny 1->32->1 (psi_mag) and 2->32->2 (psi_phase) GELU MLPs ->
    residual add of the polarized vector.
  - decode: gelu(zr @ op_w1 + op_b1) @ op_w2 + op_b2.

Sharding: data-parallel over batch (B=8 -> 1 batch element per NeuronCore).

Per-core layout: channels d (256) on partitions as two d-tiles of 128,
sequence s (1024) along the free dimension.  The causal cumsum is a native
DVE tensor_tensor_scan along the free dim (fp32).  The tiny MLPs run on the
tensor engine in bf16 with block-diagonal "selector" weight patterns: 4
elements per PE column, full 128-row output (L1 expands 4 channels x 32
hidden; L2 reduces back with an identity-aligned M=32 output so psum
accumulates a dense [128, n] delta).  The embedding and decode matmuls use
3-term bf16-split accumulation (hi/lo) for near-fp32 precision.
sqrt/recip/rsqrt are computed via exp/log (the ACT Reciprocal/Rsqrt tables
are banned for accuracy, and exp+ln share one ACT table set).

The module builder supports emitting the whole compute body `reps` times
(state is recomputed from DRAM each rep), used to measure on-device time by
wall-clock differencing through the high-overhead axon RPC path.
"""

import os

import ml_dtypes
import numpy as np

import concourse.bacc as bacc
import concourse.bass as bass
import concourse.mybir as mybir
import concourse.tile as tile
from concourse.bass_utils import run_bass_kernel_spmd

F32 = mybir.dt.float32
BF16 = mybir.dt.bfloat16
AF = mybir.ActivationFunctionType
ALU = mybir.AluOpType
NPBF = ml_dtypes.bfloat16

B, S, D, H, IN, L = 8, 1024, 256, 32, 64, 4
NCORES = 8
T = 2          # d-tiles of 128 partitions
NBLK = 2       # 512-column blocks of the free (s) dim
EPS_MAG = 1e-6

_BUILT = {}         # reps -> Bass module
LAST_RESULT = None  # BassKernelResults of the most recent run (for profiling)


def _build_module(reps=1):
    """Emit the Bass/Tile IR (shapes only; weights arrive via DRAM)."""
    nc = bacc.Bacc("TRN2", debug=False, num_devices=NCORES)

    dram = {}

    def din(name, shape, dt=F32):
        dram[name] = nc.dram_tensor(name, shape, dt, kind="ExternalInput")
        return dram[name]

    din("xaug_h", [IN + 1, S], BF16)
    din("xaug_l", [IN + 1, S], BF16)
    din("c_embw_rh", [IN + 1, D], BF16)
    din("c_embw_rl", [IN + 1, D], BF16)
    din("c_embw_ih", [IN + 1, D], BF16)
    din("c_embw_il", [IN + 1, D], BF16)
    din("c_rot_c", [128, T * S])
    din("c_rot_s", [128, T * S])
    din("c_invcnt", [128, S])
    din("c_w1m", [128, L * 1024], BF16)
    din("c_w1pa", [128, L * 1024], BF16)
    din("c_w1pc", [128, L * 1024], BF16)
    din("c_w2m", [128, L * 256], BF16)
    din("c_w2u", [128, L * 256], BF16)
    din("c_w2v", [128, L * 256], BF16)
    din("c_gbias", [128, 2 * L])
    din("c_scal", [1, 16])  # b2l per layer, bu/bv per layer, op_b2, eps
    din("c_dec1h", [128, T * H], BF16)
    din("c_dec1l", [128, T * H], BF16)
    din("c_dec2h", [H, 1], BF16)
    din("c_dec2l", [H, 1], BF16)
    din("c_decb1", [H, 1])
    out_dram = nc.dram_tensor("out", [1, S], F32, kind="ExternalOutput")

    with tile.TileContext(nc) as tc:
        with tc.tile_pool(name="persist", bufs=1) as persist:
            # ---- persistent constants ----
            invcnt = persist.tile([128, S], F32)
            nc.sync.dma_start(out=invcnt, in_=dram["c_invcnt"].ap())
            w2m = persist.tile([128, L * 256], BF16)
            nc.sync.dma_start(out=w2m, in_=dram["c_w2m"].ap())
            w2u = persist.tile([128, L * 256], BF16)
            nc.sync.dma_start(out=w2u, in_=dram["c_w2u"].ap())
            w2v = persist.tile([128, L * 256], BF16)
            nc.sync.dma_start(out=w2v, in_=dram["c_w2v"].ap())
            gbias = persist.tile([128, 2 * L], F32)
            nc.sync.dma_start(out=gbias, in_=dram["c_gbias"].ap())
            dec1h = persist.tile([128, T * H], BF16)
            nc.sync.dma_start(out=dec1h, in_=dram["c_dec1h"].ap())
            dec1l = persist.tile([128, T * H], BF16)
            nc.sync.dma_start(out=dec1l, in_=dram["c_dec1l"].ap())
            dec2h = persist.tile([H, 1], BF16)
            nc.sync.dma_start(out=dec2h, in_=dram["c_dec2h"].ap())
            dec2l = persist.tile([H, 1], BF16)
            nc.sync.dma_start(out=dec2l, in_=dram["c_dec2l"].ap())
            decb1 = persist.tile([H, 1], F32)
            nc.sync.dma_start(out=decb1, in_=dram["c_decb1"].ap())
            xh = persist.tile([IN + 1, S], BF16)
            nc.sync.dma_start(out=xh, in_=dram["xaug_h"].ap())
            xl = persist.tile([IN + 1, S], BF16)
            nc.sync.dma_start(out=xl, in_=dram["xaug_l"].ap())
            ewrh = persist.tile([IN + 1, D], BF16)
            nc.sync.dma_start(out=ewrh, in_=dram["c_embw_rh"].ap())
            ewrl = persist.tile([IN + 1, D], BF16)
            nc.sync.dma_start(out=ewrl, in_=dram["c_embw_rl"].ap())
            ewih = persist.tile([IN + 1, D], BF16)
            nc.sync.dma_start(out=ewih, in_=dram["c_embw_ih"].ap())
            ewil = persist.tile([IN + 1, D], BF16)
            nc.sync.dma_start(out=ewil, in_=dram["c_embw_il"].ap())
            rot_c = persist.tile([128, T * S], F32)
            nc.sync.dma_start(out=rot_c, in_=dram["c_rot_c"].ap())
            rot_s = persist.tile([128, T * S], F32)
            nc.sync.dma_start(out=rot_s, in_=dram["c_rot_s"].ap())

            # broadcast row of c_scal to 128 partitions for bias APs
            scal_b = persist.tile([128, 16], F32)
            nc.sync.dma_start(
                out=scal_b,
                in_=bass.AP(
                    tensor=dram["c_scal"].ap().tensor,
                    offset=dram["c_scal"].ap().offset,
                    ap=[[0, 128], [1, 16]],
                ),
            )

            # ---- state ----
            zr = [persist.tile([128, S], F32, name=f"zr{t}") for t in range(T)]
            zi = [persist.tile([128, S], F32, name=f"zi{t}") for t in range(T)]

            with tc.tile_pool(name="w1pool", bufs=2) as w1pool, \
                 tc.tile_pool(name="work", bufs=1) as work, \
                 tc.tile_pool(name="hsb", bufs=2) as hsb, \
                 tc.tile_pool(name="allt", bufs=1) as allt, \
                 tc.tile_pool(name="psh", bufs=1, space="PSUM") as psh, \
                 tc.tile_pool(name="psacc", bufs=1, space="PSUM") as psacc:

                for _rep in range(reps):
                    _emit_body(
                        nc, tc, dram, out_dram,
                        invcnt, w2m, w2u, w2v, gbias, scal_b,
                        dec1h, dec1l, dec2h, dec2l, decb1,
                        xh, xl, ewrh, ewrl, ewih, ewil, rot_c, rot_s,
                        zr, zi, w1pool, work, hsb, allt, psh, psacc,
                    )

    nc.compile()
    return nc


def _emit_body(nc, tc, dram, out_dram,
               invcnt, w2m, w2u, w2v, gbias, scal_b,
               dec1h, dec1l, dec2h, dec2l, decb1,
               xh, xl, ewrh, ewrl, ewih, ewil, rot_c, rot_s,
               zr, zi, w1pool, work, hsb, allt, psh, psacc):
    # ---- embedding + rotation (3-term bf16-split matmuls) ----
    for t in range(T):
        dcol = slice(128 * t, 128 * t + 128)
        for n in range(NBLK):
            cs = slice(512 * n, 512 * n + 512)
            tcs = slice(S * t + 512 * n, S * t + 512 * n + 512)
            ps_er = psh.tile([128, 512], F32, tag="hm", bufs=2, name="ps_er")
            ps_ei = psh.tile([128, 512], F32, tag="hp", bufs=3, name="ps_ei")
            for ps, wh, wl in ((ps_er, ewrh, ewrl), (ps_ei, ewih, ewil)):
                nc.tensor.matmul(ps, wh[:, dcol], xh[:, cs],
                                 start=True, stop=False)
                nc.tensor.matmul(ps, wh[:, dcol], xl[:, cs],
                                 start=False, stop=False)
                nc.tensor.matmul(ps, wl[:, dcol], xh[:, cs],
                                 start=False, stop=True)
            t1 = work.tile([128, 512], F32, tag="embt1", bufs=2, name="t1")
            t2 = work.tile([128, 512], F32, tag="embt2", bufs=2, name="t2")
            nc.vector.tensor_tensor(out=t1, in0=ps_er, in1=rot_c[:, tcs], op=ALU.mult)
            nc.vector.tensor_tensor(out=t2, in0=ps_ei, in1=rot_s[:, tcs], op=ALU.mult)
            nc.vector.tensor_tensor(out=zr[t][:, cs], in0=t1, in1=t2, op=ALU.subtract)
            nc.vector.tensor_tensor(out=t1, in0=ps_er, in1=rot_s[:, tcs], op=ALU.mult)
            nc.vector.tensor_tensor(out=t2, in0=ps_ei, in1=rot_c[:, tcs], op=ALU.mult)
            nc.vector.tensor_tensor(out=zi[t][:, cs], in0=t1, in1=t2, op=ALU.add)

    # ---- layers ----
    for l in range(L):
        w1m = w1pool.tile([128, 1024], BF16, tag="w1m", name="w1m")
        nc.sync.dma_start(out=w1m, in_=dram["c_w1m"].ap()[:, 1024 * l:1024 * l + 1024])
        w1pa = w1pool.tile([128, 1024], BF16, tag="w1pa", name="w1pa")
        nc.sync.dma_start(out=w1pa, in_=dram["c_w1pa"].ap()[:, 1024 * l:1024 * l + 1024])
        w1pc = w1pool.tile([128, 1024], BF16, tag="w1pc", name="w1pc")
        nc.sync.dma_start(out=w1pc, in_=dram["c_w1pc"].ap()[:, 1024 * l:1024 * l + 1024])

        lmf = []   # fp32 log-magnitude (for lmo)
        lmb = []   # bf16 copies for matmul rhs
        ppb = []
        qqb = []
        # ---- phase A: causal mean, magnitude, unit phase ----
        for t in range(T):
            Ar = work.tile([128, S], F32, tag="Ar", bufs=2, name="Ar")
            Ai = work.tile([128, S], F32, tag="Ai", bufs=2, name="Ai")
            sq = work.tile([128, S], F32, tag="sq", bufs=2, name="sq")
            tb = work.tile([128, S], F32, tag="tb", bufs=2, name="tb")
            lmt = work.tile([128, S], F32, tag=f"lm{t}", name="lmt")
            nc.vector.tensor_tensor_scan(
                out=Ar, data0=zr[t], data1=zr[t],
                initial=0.0, op0=ALU.add, op1=ALU.bypass,
            )
            nc.vector.tensor_tensor(out=Ar, in0=Ar, in1=invcnt, op=ALU.mult)
            nc.vector.tensor_tensor_scan(
                out=Ai, data0=zi[t], data1=zi[t],
                initial=0.0, op0=ALU.add, op1=ALU.bypass,
            )
            nc.vector.tensor_tensor(out=Ai, in0=Ai, in1=invcnt, op=ALU.mult)
            nc.vector.tensor_tensor(out=sq, in0=Ar, in1=Ar, op=ALU.mult)
            nc.vector.tensor_tensor(out=tb, in0=Ai, in1=Ai, op=ALU.mult)
            nc.vector.tensor_tensor(out=sq, in0=sq, in1=tb, op=ALU.add)
            # mag = exp(0.5*ln(m2)); lm = ln(mag+eps); inv = exp(-lm)
            nc.scalar.activation(tb, sq, AF.Ln)
            nc.scalar.activation(sq, tb, AF.Exp, scale=0.5)
            nc.scalar.activation(lmt, sq, AF.Ln, bias=scal_b[:, 13:14])
            nc.scalar.activation(tb, lmt, AF.Exp, scale=-1.0)
            lmtb = work.tile([128, S], BF16, tag=f"lmb{t}", name="lmtb")
            nc.vector.tensor_copy(out=lmtb, in_=lmt)
            pt = work.tile([128, S], BF16, tag=f"pb{t}", name="pt")
            nc.vector.tensor_tensor(out=pt, in0=Ar, in1=tb, op=ALU.mult)
            qt = work.tile([128, S], BF16, tag=f"qb{t}", name="qt")
            nc.vector.tensor_tensor(out=qt, in0=Ai, in1=tb, op=ALU.mult)
            lmf.append(lmt)
            lmb.append(lmtb)
            ppb.append(pt)
            qqb.append(qt)

        u_all = allt.tile([128, T * S], F32, tag="u_all", name="u_all")
        v_all = allt.tile([128, T * S], F32, tag="v_all", name="v_all")
        lmo_all = allt.tile([128, T * S], F32, tag="lmo_all", name="lmo_all")
        nn_all = allt.tile([128, T * S], F32, tag="nn_all", name="nn_all")

        # ---- phase B: the two tiny MLPs via PE (bf16) ----
        for t in range(T):
            for n in range(NBLK):
                blk = slice(512 * (2 * t + n), 512 * (2 * t + n) + 512)
                cs = slice(512 * n, 512 * n + 512)
                ps_d = psacc.tile([128, 512], F32, tag="d", name="ps_d")
                ps_u = psacc.tile([128, 512], F32, tag="u", name="ps_u")
                ps_v = psacc.tile([128, 512], F32, tag="v", name="ps_v")
                def flush_p(unit):
                    hp, rs, g = unit
                    sp = hsb.tile([128, 512], BF16, tag="sp", bufs=6, name="sp")
                    nc.scalar.activation(sp, hp, AF.Gelu, bias=gbias[:, 2 * l + 1:2 * l + 2])
                    w2c = slice(256 * l + 32 * g, 256 * l + 32 * g + 32)
                    nc.tensor.matmul(
                        ps_u[rs, :], w2u[:, w2c], sp,
                        start=(g == 0), stop=(g == 7),
                        skip_group_check=True,
                        tile_position=(0, rs.start),
                    )
                    nc.tensor.matmul(
                        ps_v[rs, :], w2v[:, w2c], sp,
                        start=(g == 0), stop=(g == 7),
                        skip_group_check=True,
                        tile_position=(0, rs.start),
                    )

                def flush_m(unit):
                    hm, rs, g = unit
                    sm = hsb.tile([128, 512], BF16, tag="sm", bufs=6, name="sm")
                    nc.scalar.activation(sm, hm, AF.Gelu, bias=gbias[:, 2 * l:2 * l + 1])
                    w2c = slice(256 * l + 32 * g, 256 * l + 32 * g + 32)
                    nc.tensor.matmul(
                        ps_d[rs, :], w2m[:, w2c], sm,
                        start=(g == 0), stop=(g == 7),
                        skip_group_check=True,
                        tile_position=(0, rs.start),
                    )

                # r-outer / g-inner: keeps each strip's 8 accumulating L2
                # matmuls close together and the hm/hp psum rings local.
                # (The g-outer strip-rotation variant measured 1.55ms vs
                # 1.08ms for this order — row-group rotation is a net loss.)
                pend_m = []
                pend_p = []
                for r in range(4):
                    rs = slice(32 * r, 32 * r + 32)
                    for g in range(8):
                        wcol = slice(128 * g, 128 * g + 128)
                        hm = psh.tile([128, 512], F32, tag="hm", bufs=2, name="hm")
                        hp = psh.tile([128, 512], F32, tag="hp", bufs=3, name="hp")
                        nc.tensor.matmul(
                            hm, w1m[rs, wcol],
                            lmb[t][rs, cs], start=True, stop=True,
                            tile_position=(32 * r, 0),
                        )
                        nc.tensor.matmul(
                            hp, w1pa[rs, wcol],
                            ppb[t][rs, cs], start=True, stop=False,
                            tile_position=(32 * r, 0),
                        )
                        nc.tensor.matmul(
                            hp, w1pc[rs, wcol],
                            qqb[t][rs, cs], start=False, stop=True,
                            tile_position=(32 * r, 0),
                        )
                        pend_m.append((hm, rs, g))
                        pend_p.append((hp, rs, g))
                        if len(pend_p) >= 3:
                            flush_p(pend_p.pop(0))
                        if len(pend_m) >= 2:
                            flush_m(pend_m.pop(0))
                for unit in pend_p:
                    flush_p(unit)
                for unit in pend_m:
                    flush_m(unit)
                # drain psums to SBUF (+tiny-MLP output biases)
                nc.vector.tensor_scalar(
                    out=u_all[:, blk], in0=ps_u,
                    scalar1=scal_b[:, 4 + l:4 + l + 1], scalar2=None, op0=ALU.add,
                )
                nc.vector.tensor_scalar(
                    out=v_all[:, blk], in0=ps_v,
                    scalar1=scal_b[:, 8 + l:8 + l + 1], scalar2=None, op0=ALU.add,
                )
                nc.vector.scalar_tensor_tensor(
                    out=lmo_all[:, blk], in0=ps_d, scalar=1.0,
                    in1=lmf[t][:, cs], op0=ALU.mult, op1=ALU.add,
                )
                nsq = work.tile([128, 512], F32, tag="nsq", bufs=2, name="nsq")
                nc.vector.tensor_tensor(out=nn_all[:, blk], in0=u_all[:, blk], in1=u_all[:, blk], op=ALU.mult)
                nc.vector.tensor_tensor(out=nsq, in0=v_all[:, blk], in1=v_all[:, blk], op=ALU.mult)
                nc.vector.tensor_tensor(out=nn_all[:, blk], in0=nn_all[:, blk], in1=nsq, op=ALU.add)

        # ---- layer tail: r/nrm and residual update ----
        # ln(n2) in place of nn_all; rin in place of lmo_all
        nc.scalar.activation(nn_all, nn_all, AF.Ln)
        nc.vector.scalar_tensor_tensor(
            out=lmo_all, in0=nn_all, scalar=-0.5,
            in1=lmo_all, op0=ALU.mult, op1=ALU.add,
        )
        rin_all = lmo_all
        # rin = exp(lm + delta + b2l - 0.5*ln(n2)) = r / nrm
        nc.scalar.activation(rin_all, lmo_all, AF.Exp, bias=scal_b[:, l:l + 1])
        for t in range(T):
            tcs = slice(S * t, S * t + S)
            tmp = work.tile([128, S], F32, tag="updt", bufs=2, name="tmp")
            nc.vector.tensor_tensor(out=tmp, in0=rin_all[:, tcs], in1=u_all[:, tcs], op=ALU.mult)
            nc.vector.tensor_tensor(out=zr[t], in0=zr[t], in1=tmp, op=ALU.add)
            nc.vector.tensor_tensor(out=tmp, in0=rin_all[:, tcs], in1=v_all[:, tcs], op=ALU.mult)
            nc.vector.tensor_tensor(out=zi[t], in0=zi[t], in1=tmp, op=ALU.add)

    # ---- decode (3-term bf16 splits) ----
    zrh = [work.tile([128, S], BF16, tag=f"zrh{t}", name=f"zrh{t}") for t in range(T)]
    zrl = [work.tile([128, S], BF16, tag=f"zrl{t}", name=f"zrl{t}") for t in range(T)]
    for t in range(T):
        nc.vector.tensor_copy(out=zrh[t], in_=zr[t])
        nc.vector.tensor_tensor(out=zrl[t], in0=zr[t], in1=zrh[t], op=ALU.subtract)
    hd = work.tile([H, S], F32, tag="hd", name="hd")
    for n in range(NBLK):
        cs = slice(512 * n, 512 * n + 512)
        ps_dec = psh.tile([H, 512], F32, tag="hm", bufs=2, name="ps_dec")
        for t in range(T):
            hcol = slice(H * t, H * t + H)
            nc.tensor.matmul(ps_dec, dec1h[:, hcol], zrh[t][:, cs],
                             start=(t == 0), stop=False)
            nc.tensor.matmul(ps_dec, dec1h[:, hcol], zrl[t][:, cs],
                             start=False, stop=False)
            nc.tensor.matmul(ps_dec, dec1l[:, hcol], zrh[t][:, cs],
                             start=False, stop=(t == T - 1))
        nc.scalar.activation(hd[:, cs], ps_dec, AF.Gelu, bias=decb1)
    hdh = work.tile([H, S], BF16, tag="hdh", name="hdh")
    hdl = work.tile([H, S], BF16, tag="hdl", name="hdl")
    nc.vector.tensor_copy(out=hdh, in_=hd)
    nc.vector.tensor_tensor(out=hdl, in0=hd, in1=hdh, op=ALU.subtract)
    preds = work.tile([1, S], F32, tag="preds", name="preds")
    for n in range(NBLK):
        cs = slice(512 * n, 512 * n + 512)
        ps_out = psh.tile([1, 512], F32, tag="hp", bufs=3, name="ps_out")
        nc.tensor.matmul(ps_out, dec2h, hdh[:, cs], start=True, stop=False)
        nc.tensor.matmul(ps_out, dec2h, hdl[:, cs], start=False, stop=False)
        nc.tensor.matmul(ps_out, dec2l, hdh[:, cs], start=False, stop=True)
        nc.scalar.activation(preds[:, cs], ps_out, AF.Identity, bias=scal_b[0:1, 12:13])
    nc.sync.dma_start(out=out_dram.ap(), in_=preds)


def _split_bf16(a):
    hi = a.astype(NPBF)
    lo = (a - hi.astype(np.float32)).astype(NPBF)
    return hi, lo


def _prep_consts(inputs):
    """Build all weight-derived constant arrays (host side, numpy)."""
    f32 = np.float32
    er_w = np.asarray(inputs["er_w"], f32)
    er_b = np.asarray(inputs["er_b"], f32)
    ei_w = np.asarray(inputs["ei_w"], f32)
    ei_b = np.asarray(inputs["ei_b"], f32)
    pm_w1 = np.asarray(inputs["pm_w1"], f32)
    pm_b1 = np.asarray(inputs["pm_b1"], f32)
    pm_w2 = np.asarray(inputs["pm_w2"], f32)
    pm_b2 = np.asarray(inputs["pm_b2"], f32)
    pp_w1 = np.asarray(inputs["pp_w1"], f32)
    pp_b1 = np.asarray(inputs["pp_b1"], f32)
    pp_w2 = np.asarray(inputs["pp_w2"], f32)
    pp_b2 = np.asarray(inputs["pp_b2"], f32)
    mag_scale = np.asarray(inputs["mag_scale"], f32)
    op_w1 = np.asarray(inputs["op_w1"], f32)
    op_b1 = np.asarray(inputs["op_b1"], f32)
    op_w2 = np.asarray(inputs["op_w2"], f32)
    op_b2 = np.asarray(inputs["op_b2"], f32)

    c = {}
    embr = np.concatenate([er_w, er_b[None, :]], axis=0)
    embi = np.concatenate([ei_w, ei_b[None, :]], axis=0)
    c["c_embw_rh"], c["c_embw_rl"] = _split_bf16(embr)
    c["c_embw_ih"], c["c_embw_il"] = _split_bf16(embi)

    pos = np.arange(S, dtype=f32)[:, None]
    freq = np.exp(-np.log(10000.0) * np.arange(D, dtype=f32) / D).astype(f32)
    theta = (pos * freq[None, :]).astype(f32)  # [S, D]
    rc = np.cos(theta).astype(f32)
    rs = np.sin(theta).astype(f32)
    rot_c = np.empty((128, T * S), f32)
    rot_s = np.empty((128, T * S), f32)
    for t in range(T):
        rot_c[:, S * t:S * t + S] = rc[:, 128 * t:128 * t + 128].T
        rot_s[:, S * t:S * t + S] = rs[:, 128 * t:128 * t + 128].T
    c["c_rot_c"] = rot_c
    c["c_rot_s"] = rot_s

    c["c_invcnt"] = np.broadcast_to(
        (1.0 / np.arange(1, S + 1, dtype=f32))[None, :], (128, S)
    ).copy()

    # L1 selector patterns: rows k in [0,32) (strip-local channel), cols
    # g*128 + (q*32+j); value = w1[j] iff k == 4g+q.  Replicated over strips.
    def l1_pack(w1_row):
        pack = np.zeros((128, L * 1024), f32)
        for l in range(L):
            pat = np.zeros((32, 1024), f32)
            for g in range(8):
                for q in range(4):
                    pat[4 * g + q, 128 * g + 32 * q:128 * g + 32 * q + 32] = w1_row[l]
            for r in range(4):
                pack[32 * r:32 * r + 32, 1024 * l:1024 * l + 1024] = pat
        return pack.astype(NPBF)

    c["c_w1m"] = l1_pack(pm_w1[:, 0, :])
    c["c_w1pa"] = l1_pack(pp_w1[:, 0, :])
    c["c_w1pc"] = l1_pack(pp_w1[:, 1, :])

    # L2 patterns: rows (q*32+j), cols l*256 + g*32 + mo; value w2[j] iff mo==4g+q
    def l2_pack(w2_col):
        pack = np.zeros((128, L * 256), f32)
        for l in range(L):
            for g in range(8):
                for q in range(4):
                    mo = 4 * g + q
                    pack[32 * q:32 * q + 32, 256 * l + 32 * g + mo] = w2_col[l]
        return pack.astype(NPBF)

    c["c_w2m"] = l2_pack(pm_w2[:, :, 0] * mag_scale[:, None])
    c["c_w2u"] = l2_pack(pp_w2[:, :, 0])
    c["c_w2v"] = l2_pack(pp_w2[:, :, 1])

    gb = np.zeros((128, 2 * L), f32)
    for l in range(L):
        for q in range(4):
            gb[32 * q:32 * q + 32, 2 * l] = pm_b1[l]
            gb[32 * q:32 * q + 32, 2 * l + 1] = pp_b1[l]
    c["c_gbias"] = gb

    scal = np.zeros((1, 16), f32)
    scal[0, 0:4] = mag_scale * pm_b2[:, 0]      # exp bias per layer
    scal[0, 4:8] = pp_b2[:, 0]                  # u bias per layer
    scal[0, 8:12] = pp_b2[:, 1]                 # v bias per layer
    scal[0, 12] = op_b2[0]
    scal[0, 13] = EPS_MAG
    c["c_scal"] = scal

    dec1 = np.zeros((128, T * H), f32)
    for t in range(T):
        dec1[:, H * t:H * t + H] = op_w1[128 * t:128 * t + 128, :]
    c["c_dec1h"], c["c_dec1l"] = _split_bf16(dec1)
    c["c_dec2h"], c["c_dec2l"] = _split_bf16(op_w2.astype(f32))
    c["c_decb1"] = op_b1[:, None].astype(f32)
    return c


def _get_built(reps=1):
    if reps not in _BUILT:
        _BUILT[reps] = _build_module(reps)
    return _BUILT[reps]


def _make_in_maps(inputs):
    consts = _prep_consts(inputs)
    x = np.asarray(inputs["x"], np.float32)  # [B, S, IN]
    in_maps = []
    for b in range(NCORES):
        m = dict(consts)
        xaug = np.empty((IN + 1, S), np.float32)
        xaug[:IN, :] = x[b].T
        xaug[IN, :] = 1.0
        m["xaug_h"], m["xaug_l"] = _split_bf16(xaug)
        in_maps.append(m)
    return in_maps


def kernel(**inputs):
    nc = _get_built()
    in_maps = _make_in_maps(inputs)

    global LAST_RESULT
    trace = bool(int(os.environ.get("KERNEL_TRACE", "0")))
    res = run_bass_kernel_spmd(
        nc, in_maps, core_ids=list(range(NCORES)), trace=trace,
    )
    LAST_RESULT = res

    out = np.empty((B, S, 1), np.float32)
    for b in range(NCORES):
        out[b, :, 0] = res.results[b]["out"][0]
    return out



# revision 27
# speedup vs baseline: 26.0212x; 26.0212x over previous
"""Trainium2 Bass kernel for nn_CVKANTimeSeries (polynomial rewrite).

Reference computation (per batch element b, sequence s, channel d):
  - complex embedding zr/zi = x @ er_w/ei_w + bias, rotated by positional
    phases (cos/sin tables).
  - 4 stacked "polarizing" layers: causal cumulative mean -> magnitude/phase
    -> tiny 1->32->1 (psi_mag) and 2->32->2 (psi_phase) GELU MLPs ->
    residual add of the polarized vector.
  - decode: gelu(zr @ op_w1 + op_b1) @ op_w2 + op_b2.

Key algorithmic observation: both tiny MLPs are *scalar* functions.
  psi_mag:  log_mag_out - log_mag = f_l(log_mag), a fixed smooth 1-D
            function per layer -> fit a degree-8 polynomial P_l(L) of
            L = ln(mag^2) = ln(Sr^2+Si^2) - 2 ln(count) with
            P_l = u + mag_scale*f_l(u), u = ln(exp(L/2)+1e-6).
  psi_phase: acts on the unit vector (cos phi, sin phi); its UN-normalized
            output v(phi) has Fourier content that dies at harmonic 2
            (gelu of ~0.14-amplitude args), so
              v_c(phi) = a0 + a1 cos + b1 sin + a2 cos2 + b2 sin2
            with cos2 = (Sr^2-Si^2)/|S|^2, sin2 = 2 Sr Si/|S|^2.
            The L2 normalization runs on-device through the exp/ln trick:
            r_hat = exp(P_l(L) - 0.5 ln|v|^2), z += r_hat * v.
Counts cancel in the phase (p = Sr/|S|), so invcnt tables disappear.

This removes ALL layer matmuls and ALL Gelu activations (the baseline's
bottleneck: ACT engine 83% busy on 1026 gelu ops + table thrash).  The
only ACT funcs used in layers are Ln/Exp/Square - one table set
(natural_log_exp_and_others), zero table switches.  Per layer:
4 DVE scans, ~26 tensor-tensor/stt ops, 3 tensor-scalar, 9 ACT ops.
Elementwise ops are column-sliced DVE : GPSIMD/Pool ~ 1408 : 640 so both
engines finish together (Pool runs TT at 0.42 efficiency).

Coefficients are fitted at runtime from the actual weights (host-side,
numpy-only) and baked into the module as float immediates; the module
cache is keyed on them.

Sharding: data-parallel over batch (B=8 -> 1 batch element per core).
Per-core layout: channels d (256) as two partition tiles of 128 stored
side by side in the free dim ([128, 2048]); sequence s along free dim.
Embedding and decode stay on the tensor engine with 3-term bf16-split
accumulation (near-fp32).
"""

import math
import os

import ml_dtypes
import numpy as np

import concourse.bacc as bacc
import concourse.bass as bass
import concourse.mybir as mybir
import concourse.tile as tile
from concourse.bass_utils import run_bass_kernel_spmd

F32 = mybir.dt.float32
BF16 = mybir.dt.bfloat16
AF = mybir.ActivationFunctionType
ALU = mybir.AluOpType
NPBF = ml_dtypes.bfloat16

B, S, D, H, IN, L = 8, 1024, 256, 32, 64, 4
NCORES = 8
T = 2               # d-tiles of 128 partitions
NBLK = 2            # 512-column blocks for embedding/decode matmuls
FREE = T * S        # 2048 columns: the two d-tiles side by side
SPL = int(os.environ.get("KERNEL_SPL", "512"))  # DVE columns per 2048 of TT ops
HALVES = bool(int(os.environ.get("KERNEL_HALVES", "1")))
NM = 8              # magnitude polynomial degree
LLO, LHI = -26.0, 10.0   # fit range for L = ln(mag^2)
EPS_MAG = 1e-6

_BUILT = {}         # (coeffs, reps) -> Bass module
LAST_RESULT = None  # BassKernelResults of the most recent run (for profiling)
LAST_COEFFS = None  # set by _prep_consts; used by _get_built for test.py


def _sliced(fv, fp, *aps, whole=False):
    """Emit an elementwise op column-sliced across DVE (fv) and Pool (fp)."""
    if whole:
        fv(*[a for a in aps])
        return
    fv(*[a[:, :SPL] for a in aps])
    fp(*[a[:, SPL:] for a in aps])


def _build_module(coeffs, reps=1):
    """Emit the Bass/Tile IR. `coeffs` carries all weight-derived immediates:
    (alpha, ((mono...), (au...), (av...)) x L, op_b2)."""
    alpha, per_layer, op_b2 = coeffs
    nc = bacc.Bacc("TRN2", debug=False, num_devices=NCORES)

    dram = {}

    def din(name, shape, dt=F32):
        dram[name] = nc.dram_tensor(name, shape, dt, kind="ExternalInput")
        return dram[name]

    din("xaug_h", [IN + 1, S], BF16)
    din("xaug_l", [IN + 1, S], BF16)
    din("c_embw_rh", [IN + 1, D], BF16)
    din("c_embw_rl", [IN + 1, D], BF16)
    din("c_embw_ih", [IN + 1, D], BF16)
    din("c_embw_il", [IN + 1, D], BF16)
    din("c_rot_c", [128, FREE])
    din("c_rot_s", [128, FREE])
    din("c_lncnt2p", [128, FREE])
    din("c_scal", [1, 8])  # per-layer exp bias mono[0], op_b2
    din("c_dec1h", [128, T * H], BF16)
    din("c_dec1l", [128, T * H], BF16)
    din("c_dec2h", [H, 1], BF16)
    din("c_dec2l", [H, 1], BF16)
    din("c_decb1", [H, 1])
    out_dram = nc.dram_tensor("out", [1, S], F32, kind="ExternalOutput")

    with tile.TileContext(nc) as tc:
        # Pre-load the combined Ln/Exp/Square ACT table set: the automatic
        # pass would otherwise thrash natural_log <-> exp_and_others on
        # every Ln->Exp transition (4 loads x ~1.3us per layer).
        from concourse.hw_specs import get_activation_tables
        tabs = list(get_activation_tables(nc.m.arch).items())
        cid = [i for i, (nm, _) in enumerate(tabs)
               if nm == "natural_log_exp_and_others"]
        if cid:
            ld = mybir.InstLoadActFuncSet(
                name=nc.get_next_instruction_name(), act_func_set_id=cid[0])
            ld.engine = mybir.EngineType.Activation
            nc.scalar.add_instruction(ld)
        with tc.tile_pool(name="persist", bufs=1) as persist:
            # ---- persistent constants ----
            lncnt2p = persist.tile([128, FREE], F32)
            nc.sync.dma_start(out=lncnt2p, in_=dram["c_lncnt2p"].ap())
            rot_c = persist.tile([128, FREE], F32)
            nc.sync.dma_start(out=rot_c, in_=dram["c_rot_c"].ap())
            rot_s = persist.tile([128, FREE], F32)
            nc.sync.dma_start(out=rot_s, in_=dram["c_rot_s"].ap())
            dec1h = persist.tile([128, T * H], BF16)
            nc.sync.dma_start(out=dec1h, in_=dram["c_dec1h"].ap())
            dec1l = persist.tile([128, T * H], BF16)
            nc.sync.dma_start(out=dec1l, in_=dram["c_dec1l"].ap())
            dec2h = persist.tile([H, 1], BF16)
            nc.sync.dma_start(out=dec2h, in_=dram["c_dec2h"].ap())
            dec2l = persist.tile([H, 1], BF16)
            nc.sync.dma_start(out=dec2l, in_=dram["c_dec2l"].ap())
            decb1 = persist.tile([H, 1], F32)
            nc.sync.dma_start(out=decb1, in_=dram["c_decb1"].ap())
            xh = persist.tile([IN + 1, S], BF16)
            nc.sync.dma_start(out=xh, in_=dram["xaug_h"].ap())
            xl = persist.tile([IN + 1, S], BF16)
            nc.sync.dma_start(out=xl, in_=dram["xaug_l"].ap())
            ewrh = persist.tile([IN + 1, D], BF16)
            nc.sync.dma_start(out=ewrh, in_=dram["c_embw_rh"].ap())
            ewrl = persist.tile([IN + 1, D], BF16)
            nc.sync.dma_start(out=ewrl, in_=dram["c_embw_rl"].ap())
            ewih = persist.tile([IN + 1, D], BF16)
            nc.sync.dma_start(out=ewih, in_=dram["c_embw_ih"].ap())
            ewil = persist.tile([IN + 1, D], BF16)
            nc.sync.dma_start(out=ewil, in_=dram["c_embw_il"].ap())

            # broadcast row of c_scal to 128 partitions for bias APs
            scal_b = persist.tile([128, 8], F32)
            nc.sync.dma_start(
                out=scal_b,
                in_=bass.AP(
                    tensor=dram["c_scal"].ap().tensor,
                    offset=dram["c_scal"].ap().offset,
                    ap=[[0, 128], [1, 8]],
                ),
            )

            # ---- state ----
            zr = persist.tile([128, FREE], F32, name="zr")
            zi = persist.tile([128, FREE], F32, name="zi")

            with tc.tile_pool(name="work", bufs=1) as work, \
                 tc.tile_pool(name="psh", bufs=1, space="PSUM") as psh:
                for _rep in range(reps):
                    _emit_body(
                        nc, tc, dram, out_dram, alpha, per_layer, scal_b,
                        lncnt2p, rot_c, rot_s,
                        dec1h, dec1l, dec2h, dec2l, decb1,
                        xh, xl, ewrh, ewrl, ewih, ewil,
                        zr, zi, work, psh,
                    )

    nc.compile()
    return nc


def _emit_body(nc, tc, dram, out_dram, alpha, per_layer, scal_b,
               lncnt2p, rot_c, rot_s,
               dec1h, dec1l, dec2h, dec2l, decb1,
               xh, xl, ewrh, ewrl, ewih, ewil,
               zr, zi, work, psh):
    # The real ISA runs tensor_scalar/scalar_tensor_tensor/scans ONLY on DVE
    # (GPSIMD has tensor_tensor ucode but no TensorScalarPtr support), so:
    #   - plain TT ops: column-split DVE:Pool = SPL:(FREE-SPL)
    #   - stt / tensor_scalar / scans: DVE
    # With HALVES the two d-tiles are emitted as separate [*,1024] pieces so
    # tile-0's chain can run while tile-1 is still scanning.
    if HALVES:
        sph = SPL // 2
        tt_pieces = [(slice(0, sph), slice(sph, S)),
                     (slice(S, S + sph), slice(S + sph, FREE))]
        cols = [slice(0, S), slice(S, FREE)]
    else:
        tt_pieces = [(slice(0, SPL), slice(SPL, FREE))]
        cols = [slice(0, FREE)]

    def v_tt(out, in0, in1, op):
        for dv, pl in tt_pieces:
            if dv.stop > dv.start:
                nc.vector.tensor_tensor(out=out[:, dv], in0=in0[:, dv],
                                        in1=in1[:, dv], op=op)
            nc.gpsimd.tensor_tensor(out=out[:, pl], in0=in0[:, pl],
                                    in1=in1[:, pl], op=op)

    def v_stt(out, in0, scalar, in1, op0, op1):
        for cs in cols:
            nc.vector.scalar_tensor_tensor(
                out=out[:, cs], in0=in0[:, cs], scalar=scalar,
                in1=in1[:, cs], op0=op0, op1=op1)

    def v_ts(out, in0, s1, s2, op0, op1=None):
        for cs in cols:
            if op1 is None:
                nc.vector.tensor_scalar(out=out[:, cs], in0=in0[:, cs],
                                        scalar1=s1, scalar2=None, op0=op0)
            else:
                nc.vector.tensor_scalar(out=out[:, cs], in0=in0[:, cs],
                                        scalar1=s1, scalar2=s2, op0=op0, op1=op1)

    def v_act(out, in_, func, **kw):
        for cs in cols:
            nc.scalar.activation(out[:, cs], in_[:, cs], func, **kw)

    # ---- embedding + rotation (3-term bf16-split matmuls) ----
    for t in range(T):
        dcol = slice(128 * t, 128 * t + 128)
        for n in range(NBLK):
            cs = slice(512 * n, 512 * n + 512)
            tcs = slice(S * t + 512 * n, S * t + 512 * n + 512)
            ps_er = psh.tile([128, 512], F32, tag="pser", bufs=2, name="ps_er")
            ps_ei = psh.tile([128, 512], F32, tag="psei", bufs=2, name="ps_ei")
            for ps, wh, wl in ((ps_er, ewrh, ewrl), (ps_ei, ewih, ewil)):
                nc.tensor.matmul(ps, wh[:, dcol], xh[:, cs],
                                 start=True, stop=False)
                nc.tensor.matmul(ps, wh[:, dcol], xl[:, cs],
                                 start=False, stop=False)
                nc.tensor.matmul(ps, wl[:, dcol], xh[:, cs],
                                 start=False, stop=True)
            t1 = work.tile([128, 512], F32, tag="embt1", bufs=2, name="t1")
            t2 = work.tile([128, 512], F32, tag="embt2", bufs=2, name="t2")
            t1b = work.tile([128, 512], F32, tag="embt1b", bufs=2, name="t1b")
            t2b = work.tile([128, 512], F32, tag="embt2b", bufs=2, name="t2b")
            # rotation: zr = er*c - ei*s ; zi = er*s + ei*c
            # (GPSIMD cannot read PSUM, so the ps_* reads stay on DVE;
            # the PSUM-free combines go to Pool.)
            nc.vector.tensor_tensor(out=t1, in0=ps_er, in1=rot_c[:, tcs], op=ALU.mult)
            nc.vector.tensor_tensor(out=t2, in0=ps_ei, in1=rot_s[:, tcs], op=ALU.mult)
            nc.gpsimd.tensor_tensor(out=zr[:, tcs], in0=t1, in1=t2, op=ALU.subtract)
            nc.vector.tensor_tensor(out=t1b, in0=ps_er, in1=rot_s[:, tcs], op=ALU.mult)
            nc.vector.tensor_tensor(out=t2b, in0=ps_ei, in1=rot_c[:, tcs], op=ALU.mult)
            nc.gpsimd.tensor_tensor(out=zi[:, tcs], in0=t1b, in1=t2b, op=ALU.add)

    # ---- layers (no matmuls, no gelu; Ln/Exp/Square only) ----
    # Residual updates are scan-fused: layer l produces the residual pair
    # (ur, ui) = rh*(vu, vv); layer l+1's cumsum consumes them via the
    # scan's second data operand (state = (zr + state) + ur), so the
    # z += u materialization happens OFF the critical path during l+1.
    ur = ui = None
    for l in range(L):
        mono, cu, cv = per_layer[l]
        Sr = work.tile([128, FREE], F32, tag="Sr", name="Sr")
        Si = work.tile([128, FREE], F32, tag="Si", name="Si")
        # causal cumsums: independent per 1024-tile; zr halves on DVE,
        # zi halves on Pool so the scans overlap.
        for t in range(T):
            h = slice(S * t, S * t + S)
            if ur is None:
                nc.vector.tensor_tensor_scan(
                    out=Sr[:, h], data0=zr[:, h], data1=zr[:, h],
                    initial=0.0, op0=ALU.add, op1=ALU.bypass)
                nc.vector.tensor_tensor_scan(
                    out=Si[:, h], data0=zi[:, h], data1=zi[:, h],
                    initial=0.0, op0=ALU.add, op1=ALU.bypass)
            else:
                nc.vector.tensor_tensor_scan(
                    out=Sr[:, h], data0=zr[:, h], data1=ur[:, h],
                    initial=0.0, op0=ALU.add, op1=ALU.add)
                nc.vector.tensor_tensor_scan(
                    out=Si[:, h], data0=zi[:, h], data1=ui[:, h],
                    initial=0.0, op0=ALU.add, op1=ALU.add)
        if ur is not None:
            # fold the previous layer's residual into the state tensors
            # (consumers: this layer's magnitude/phase chain is already fed
            # by the fused scans; only the NEXT layer's scan reads zr/zi,
            # and after the last layer only zr is read, by decode).
            v_tt(zr, zr, ur, ALU.add)
            if l < L - 1:
                v_tt(zi, zi, ui, ALU.add)

        sqr = work.tile([128, FREE], F32, tag="sqr", name="sqr")
        sqi = work.tile([128, FREE], F32, tag="sqi", name="sqi")
        v_act(sqr, Sr, AF.Square)
        v_act(sqi, Si, AF.Square)
        d2 = work.tile([128, FREE], F32, tag="d2", name="d2")
        v_tt(d2, sqr, sqi, ALU.subtract)
        s2 = sqr  # in-place: sqr dead after d2/s2
        v_tt(s2, sqr, sqi, ALU.add)
        m = work.tile([128, FREE], F32, tag="m", name="m")
        v_tt(m, Sr, Si, ALU.mult)
        lam = sqi  # in-place: sqi dead after s2
        v_act(lam, s2, AF.Ln)
        inv = work.tile([128, FREE], F32, tag="inv", name="inv")
        v_act(inv, lam, AF.Exp, scale=-0.5)
        inv2 = work.tile([128, FREE], F32, tag="inv2", name="inv2")
        v_act(inv2, lam, AF.Exp, scale=-1.0)
        # t-variable for the magnitude polynomial: tv = alpha*lam - lncnt2p
        tv = lam  # in-place: lam dead after inv/inv2
        v_stt(tv, lam, float(alpha), lncnt2p, ALU.mult, ALU.subtract)
        # harmonic basis: p=cos, q=sin, c2=cos2, s2p=sin2 (sans factor 2)
        p = Sr  # in-place: Sr dead after sqr/m
        q = Si
        v_tt(p, Sr, inv, ALU.mult)
        v_tt(q, Si, inv, ALU.mult)
        c2 = d2
        v_tt(c2, d2, inv2, ALU.mult)
        s2p = m
        v_tt(s2p, m, inv2, ALU.mult)
        # magnitude polynomial P = sum_{k>=1} mono[k] t^k (Horner via stt);
        # mono[0] goes into the final Exp bias.
        P = work.tile([128, FREE], F32, tag="P", name="P")
        v_ts(P, tv, float(mono[NM]), None, ALU.mult)
        for k in range(NM - 1, 0, -1):
            v_stt(P, P, float(mono[k]), tv, ALU.add, ALU.mult)
        # phase assembly: v_c = a0 + a1*p + b1*q + a2*c2 + (2*b2)*s2p
        vu = work.tile([128, FREE], F32, tag="vu", name="vu")
        vv = work.tile([128, FREE], F32, tag="vv", name="vv")
        for vt, (a0, a1, b1, a2, b22) in ((vu, cu), (vv, cv)):
            v_ts(vt, s2p, float(b22), float(a0), ALU.mult, ALU.add)
            v_stt(vt, c2, float(a2), vt, ALU.mult, ALU.add)
            v_stt(vt, q, float(b1), vt, ALU.mult, ALU.add)
            v_stt(vt, p, float(a1), vt, ALU.mult, ALU.add)
        # normalization + residual: r_hat = exp(P + mono0 - 0.5 ln|v|^2)
        svu = work.tile([128, FREE], F32, tag="svu", name="svu")
        svv = work.tile([128, FREE], F32, tag="svv", name="svv")
        v_act(svu, vu, AF.Square)
        v_act(svv, vv, AF.Square)
        n2 = svu
        v_tt(n2, svu, svv, ALU.add)
        lam2 = svv
        v_act(lam2, n2, AF.Ln)
        E = n2
        v_stt(E, lam2, -0.5, P, ALU.mult, ALU.add)
        rh = P  # in-place: P dead after E
        v_act(rh, E, AF.Exp, bias=scal_b[:, l:l + 1])
        # residual pair for the next layer's fused scan (folded there)
        ur = work.tile([128, FREE], F32, tag="ur", name="ur")
        ui = work.tile([128, FREE], F32, tag="ui", name="ui")
        v_tt(ur, rh, vu, ALU.mult)
        v_tt(ui, rh, vv, ALU.mult)

    # fold the last layer's residual (decode only needs zr)
    v_tt(zr, zr, ur, ALU.add)

    # ---- decode (3-term bf16 splits) ----
    zrh = work.tile([128, FREE], BF16, tag="zrh", name="zrh")
    zrl = work.tile([128, FREE], BF16, tag="zrl", name="zrl")
    nc.vector.tensor_copy(out=zrh, in_=zr)
    v_tt(zrl, zr, zrh, ALU.subtract)
    hd = work.tile([H, S], F32, tag="hd", name="hd")
    for n in range(NBLK):
        cs = slice(512 * n, 512 * n + 512)
        ps_dec = psh.tile([H, 512], F32, tag="pser", bufs=2, name="ps_dec")
        for t in range(T):
            hcol = slice(H * t, H * t + H)
            tcs = slice(S * t + 512 * n, S * t + 512 * n + 512)
            nc.tensor.matmul(ps_dec, dec1h[:, hcol], zrh[:, tcs],
                             start=(t == 0), stop=False)
            nc.tensor.matmul(ps_dec, dec1h[:, hcol], zrl[:, tcs],
                             start=False, stop=False)
            nc.tensor.matmul(ps_dec, dec1l[:, hcol], zrh[:, tcs],
                             start=False, stop=(t == T - 1))
        nc.scalar.activation(hd[:, cs], ps_dec, AF.Gelu, bias=decb1)
    hdh = work.tile([H, S], BF16, tag="hdh", name="hdh")
    hdl = work.tile([H, S], BF16, tag="hdl", name="hdl")
    nc.vector.tensor_copy(out=hdh, in_=hd)
    nc.vector.tensor_tensor(out=hdl, in0=hd, in1=hdh, op=ALU.subtract)
    preds = work.tile([1, S], F32, tag="preds", name="preds")
    for n in range(NBLK):
        cs = slice(512 * n, 512 * n + 512)
        ps_out = psh.tile([1, 512], F32, tag="psei", bufs=2, name="ps_out")
        nc.tensor.matmul(ps_out, dec2h, hdh[:, cs], start=True, stop=False)
        nc.tensor.matmul(ps_out, dec2h, hdl[:, cs], start=False, stop=False)
        nc.tensor.matmul(ps_out, dec2l, hdh[:, cs], start=False, stop=True)
        nc.scalar.activation(preds[:, cs], ps_out, AF.Identity,
                             bias=scal_b[0:1, 4:5])
    nc.sync.dma_start(out=out_dram.ap(), in_=preds)


def _split_bf16(a):
    hi = a.astype(NPBF)
    lo = (a - hi.astype(np.float32)).astype(NPBF)
    return hi, lo


_erf = np.vectorize(math.erf)


def _gelu_np(x):
    return 0.5 * x * (1.0 + _erf(x / np.sqrt(2.0)))


def _fit_coeffs(inputs):
    """Fit per-layer polynomial/harmonic coefficients (float64 host math)."""
    f64 = np.float64
    pm_w1 = np.asarray(inputs["pm_w1"], f64)
    pm_b1 = np.asarray(inputs["pm_b1"], f64)
    pm_w2 = np.asarray(inputs["pm_w2"], f64)
    pm_b2 = np.asarray(inputs["pm_b2"], f64)
    pp_w1 = np.asarray(inputs["pp_w1"], f64)
    pp_b1 = np.asarray(inputs["pp_b1"], f64)
    pp_w2 = np.asarray(inputs["pp_w2"], f64)
    pp_b2 = np.asarray(inputs["pp_b2"], f64)
    mag_scale = np.asarray(inputs["mag_scale"], f64)

    alpha = 2.0 / (LHI - LLO)
    per_layer = []
    Lg = np.linspace(LLO, LHI, 4001)
    tg = (2 * Lg - (LHI + LLO)) / (LHI - LLO)
    NG = 4096
    phi = np.linspace(0, 2 * np.pi, NG, endpoint=False)
    pv = np.stack([np.cos(phi), np.sin(phi)], axis=-1)
    basis = np.stack([np.ones(NG), np.cos(phi), np.sin(phi),
                      np.cos(2 * phi), np.sin(2 * phi)], axis=-1)
    for l in range(L):
        # magnitude path: P(L) = u + mag_scale*f(u), u = ln(exp(L/2)+eps)
        u = np.log(np.exp(0.5 * Lg) + EPS_MAG)
        h = _gelu_np(u[:, None] * pm_w1[l][0] + pm_b1[l])
        target = u + mag_scale[l] * (h @ pm_w2[l][:, 0] + pm_b2[l][0])
        cf = np.polynomial.chebyshev.chebfit(tg, target, NM)
        mono = np.polynomial.chebyshev.cheb2poly(cf)
        # phase path: direction-weighted harmonic fit of the raw MLP output
        v = _gelu_np(pv @ pp_w1[l] + pp_b1[l]) @ pp_w2[l] + pp_b2[l]
        w = 1.0 / np.linalg.norm(v, axis=-1)
        comp = []
        for c in range(2):
            sol, *_ = np.linalg.lstsq(basis * w[:, None], v[:, c] * w,
                                      rcond=None)
            a0, a1, b1, a2, b2 = sol
            comp.append((float(a0), float(a1), float(b1), float(a2),
                         float(2.0 * b2)))
        per_layer.append((tuple(float(x) for x in mono),
                          tuple(comp[0]), tuple(comp[1])))
    op_b2 = float(np.asarray(inputs["op_b2"], f64)[0])
    return (float(alpha), tuple(per_layer), op_b2)


def _prep_consts(inputs):
    """Build weight-derived DRAM constant arrays + baked coefficients."""
    global LAST_COEFFS
    f32 = np.float32
    er_w = np.asarray(inputs["er_w"], f32)
    er_b = np.asarray(inputs["er_b"], f32)
    ei_w = np.asarray(inputs["ei_w"], f32)
    ei_b = np.asarray(inputs["ei_b"], f32)
    op_w1 = np.asarray(inputs["op_w1"], f32)
    op_b1 = np.asarray(inputs["op_b1"], f32)
    op_w2 = np.asarray(inputs["op_w2"], f32)

    c = {}
    embr = np.concatenate([er_w, er_b[None, :]], axis=0)
    embi = np.concatenate([ei_w, ei_b[None, :]], axis=0)
    c["c_embw_rh"], c["c_embw_rl"] = _split_bf16(embr)
    c["c_embw_ih"], c["c_embw_il"] = _split_bf16(embi)

    pos = np.arange(S, dtype=f32)[:, None]
    freq = np.exp(-np.log(10000.0) * np.arange(D, dtype=f32) / D).astype(f32)
    theta = (pos * freq[None, :]).astype(f32)  # [S, D]
    rc = np.cos(theta).astype(f32)
    rs = np.sin(theta).astype(f32)
    rot_c = np.empty((128, FREE), f32)
    rot_s = np.empty((128, FREE), f32)
    for t in range(T):
        rot_c[:, S * t:S * t + S] = rc[:, 128 * t:128 * t + 128].T
        rot_s[:, S * t:S * t + S] = rs[:, 128 * t:128 * t + 128].T
    c["c_rot_c"] = rot_c
    c["c_rot_s"] = rot_s

    # tv = alpha*Lambda - lncnt2p with lncnt2p = alpha*(2 ln(count) + mid)
    mid = (LHI + LLO) / 2.0
    alpha = 2.0 / (LHI - LLO)
    ln2c = 2.0 * np.log(np.arange(1, S + 1, dtype=np.float64))
    row = (alpha * (ln2c + mid)).astype(f32)[None, :]
    c["c_lncnt2p"] = np.broadcast_to(
        np.concatenate([row, row], axis=1), (128, FREE)).copy()

    dec1 = np.zeros((128, T * H), f32)
    for t in range(T):
        dec1[:, H * t:H * t + H] = op_w1[128 * t:128 * t + 128, :]
    c["c_dec1h"], c["c_dec1l"] = _split_bf16(dec1)
    c["c_dec2h"], c["c_dec2l"] = _split_bf16(op_w2.astype(f32))
    c["c_decb1"] = op_b1[:, None].astype(f32)

    LAST_COEFFS = _fit_coeffs(inputs)
    _, per_layer, op_b2 = LAST_COEFFS
    scal = np.zeros((1, 8), f32)
    for l in range(L):
        scal[0, l] = per_layer[l][0][0]   # mono[0] -> exp bias
    scal[0, 4] = op_b2
    c["c_scal"] = scal
    return c


def _get_built(reps=1, coeffs=None):
    if coeffs is None:
        coeffs = LAST_COEFFS
    assert coeffs is not None, "call _prep_consts/_make_in_maps first"
    key = (coeffs, reps, SPL, HALVES)
    if key not in _BUILT:
        _BUILT[key] = _build_module(coeffs, reps)
    return _BUILT[key]


def _make_in_maps(inputs):
    consts = _prep_consts(inputs)
    x = np.asarray(inputs["x"], np.float32)  # [B, S, IN]
    in_maps = []
    for b in range(NCORES):
        mday = dict(consts)
        xaug = np.empty((IN + 1, S), np.float32)
        xaug[:IN, :] = x[b].T
        xaug[IN, :] = 1.0
        mday["xaug_h"], mday["xaug_l"] = _split_bf16(xaug)
        in_maps.append(mday)
    return in_maps


def kernel(**inputs):
    in_maps = _make_in_maps(inputs)
    nc = _get_built()

    global LAST_RESULT
    trace = bool(int(os.environ.get("KERNEL_TRACE", "0")))
    res = run_bass_kernel_spmd(
        nc, in_maps, core_ids=list(range(NCORES)), trace=trace,
    )
    LAST_RESULT = res

    out = np.empty((B, S, 1), np.float32)
    for b in range(NCORES):
        out[b, :, 0] = res.results[b]["out"][0]
    return out
